# revision 9
# baseline (speedup 1.0000x reference)
"""Trainium2 Bass kernel for nn_NeuralODE_7121055776945.

Latent neural ODE: encoder MLP -> variational z0 -> 499-step fixed-step
Tsit5 solve; each of the 6 stages evaluates a 6-layer MLP (16->64x5->48),
perturbs with per-sample threefry noise, applies tanh and contracts with a
cubic-Hermite control derivative.

Strategy:
  - Pure data parallel: batch 512 -> 64 samples per core on 8 cores.
  - Feature-major on-chip layout: activations are (features, batch) tiles.
  - The RK stage combination y_s = y + h*sum_j a_sj*k_j is fused into the
    first MLP matmul: lhsT = Cs @ W1.T precomputed on host, contracting two
    128-partition "Kbuf" tiles holding y and the h*k_j stage results at
    32-aligned partition slots (compute-engine APs must start at partition
    0/32/64/96 on TRN2).
  - softplus(x) = ln(1 + e^x) via two ScalarE ops (exp, then ln with bias=1);
    both live in the natural_log_exp_and_others activation table set so the
    act table loads exactly once (this build has no native Softplus table).
  - tanh(z) = 1 - 2/(1+e^{2z}): ScalarE exp(scale=2, bias=2*b6) reading the
    matmul PSUM, VectorE +1, custom-DVE fast reciprocal (~51 ULP), then one
    fused tensor_scalar (r*-2+1).  The per-eval channel blocks live at rows
    {0,32,64} of 96-row tiles so the partition-block reduction uses legal
    32-aligned slices.  h is folded into the streamed dcontrol data
    (dcb = h*dc), so per-step h values are exact.
  - Per-sample threefry RNG (eps0 and all 2994 stage-eval eps) and the
    cubic-Hermite dcontrol values are bit-exactly precomputed on host with
    jax on CPU, replicating the reference's exact vmap/scan structure (jax
    RNG values depend on the batching structure!).

All matmuls are fp32 (bf16/tf32 fail the accuracy budget: this ODE amplifies
activation-level noise ~80x over the 499 steps).
"""

import os
import numpy as np

B, T, D, L, H = 512, 500, 2, 16, 32
STD = 1e-3
NCORES = 8
BLOC = B // NCORES  # 64
NSTEPS_FULL = T - 1  # 499

C2, C3, C4, C5 = 0.161, 0.327, 0.9, 0.9800255409045097
A_LOW = [
    [],
    [0.161],
    [-0.008480655492356989, 0.335480655492357],
    [2.8971530571054935, -6.359448489975075, 4.3622954328695815],
    [5.325864828439257, -11.748883564062828, 7.4955393428898365, -0.09249506636175525],
    [5.86145544294642, -12.92096931784711, 8.159367898576159, -0.071584973281401, -0.028269050394068383],
]
B_W = [0.09646076681806523, 0.01, 0.4798896504144996, 1.379008574103742, -3.290069515436081, 2.324710524099774]

# Kbuf slot layout: KbufA rows {0: y, 32: hk1, 64: hk2, 96: hk3},
# KbufB rows {0: hk4, 32: hk5, 64: hk6}.  All other rows zero.
SLOT_A = {0: 0, 1: 32, 2: 64, 3: 96}   # y, hk1..hk3
SLOT_B = {4: 0, 5: 32, 6: 64}          # hk4..hk6 (key = j index 1-based - 3?)


def _np(x):
    return np.asarray(x)


# --------------------------------------------------------------------------
# Host-side exact precompute (jax on CPU)
# --------------------------------------------------------------------------

def _host_precompute(xs, key, n_steps):
    """Returns (eps0 (B,L) f32, eps_all (B, n_steps, 6, L, D+1) f32,
    dc_all (n_steps, 6, B, D+1) f32, h_arr (n_steps,) f32)."""
    import jax
    import jax.numpy as jnp

    cpu = jax.devices("cpu")[0]
    with jax.default_device(cpu):
        xs_j = jnp.asarray(xs, dtype=jnp.float32)
        key_j = jnp.asarray(key)
        t_eval = jnp.linspace(0.0, 1.0, T, dtype=jnp.float32)
        dt = t_eval[1:] - t_eval[:-1]

        keys = jax.random.split(key_j, B)

        @jax.jit
        def eps0_fn(keys):
            return jax.vmap(lambda k: jax.random.normal(k, (L,)))(keys)

        eps0 = np.asarray(eps0_fn(keys), dtype=np.float32)

        t0 = t_eval[:n_steps]
        h = dt[:n_steps]

        # eps must replicate the reference's exact computation structure:
        # jax.random values differ between batched (vmap-over-keys) and
        # unbatched calls, and also depend on the vmap nesting. The reference
        # calls vmap(lambda k: normal(fold_in(k, ti), (L, D+1)))(keys) once
        # per stage inside a scan; mirror exactly that.
        @jax.jit
        def eps_scan(keys, t0s, hs):
            def body(carry, th):
                tt0, hh = th
                outs = []
                for tt in (tt0, tt0 + C2 * hh, tt0 + C3 * hh, tt0 + C4 * hh,
                           tt0 + C5 * hh, tt0 + hh):
                    ti = (tt * 1e9).astype(jnp.int32)
                    outs.append(jax.vmap(
                        lambda k: jax.random.normal(
                            jax.random.fold_in(k, ti), (L, D + 1))
                    )(keys))
                return carry, jnp.stack(outs)

            _, out = jax.lax.scan(body, 0, (t0s, hs))
            return out  # (n_steps, 6, B, L, D+1)

        eps_all = np.asarray(eps_scan(keys, t0, h), dtype=np.float32)
        eps_all = np.ascontiguousarray(np.transpose(eps_all, (2, 0, 1, 3, 4)))
        # (B, n_steps, 6, L, D+1)

        tmat = jnp.stack(
            [t0, t0 + C2 * h, t0 + C3 * h, t0 + C4 * h, t0 + C5 * h, t0 + h], axis=1
        )  # (n_steps, 6) f32

        xs_aug = jnp.concatenate(
            [jnp.broadcast_to(t_eval[None, :, None], (B, T, 1)), xs_j], axis=-1
        )
        slopes = (xs_aug[:, 1:] - xs_aug[:, :-1]) / dt[None, :, None]
        ds = jnp.concatenate([slopes[:, :1], slopes], axis=1)

        @jax.jit
        def dc_fn(ts):
            def dcontrol(t):
                i = jnp.clip(jnp.searchsorted(t_eval, t, side="right") - 1, 0, T - 2)
                hh = t_eval[i + 1] - t_eval[i]
                s = (t - t_eval[i]) / hh
                y_l = jnp.take(xs_aug, i, axis=1)
                y_r = jnp.take(xs_aug, i + 1, axis=1)
                d_l = jnp.take(ds, i, axis=1)
                d_r = jnp.take(ds, i + 1, axis=1)
                return (
                    ((6 * s * s - 6 * s) * (y_l - y_r)) / hh
                    + (3 * s * s - 4 * s + 1) * d_l
                    + (3 * s * s - 2 * s) * d_r
                )
            return jax.vmap(dcontrol)(ts)

        dc_all = np.asarray(dc_fn(tmat.reshape(-1)), dtype=np.float32)
        dc_all = dc_all.reshape(n_steps, 6, B, D + 1)
        h_arr = np.asarray(h, dtype=np.float32)

    return eps0, eps_all, dc_all, h_arr


def _stage_matrices(gen_params):
    """Fused first-layer weights.  Returns (ma_list[6] (128,64), mb_list[6]
    (128,64) or None, cfa (128,16), cfb (128,16)), fp32.
    KbufA rows: y@0, hk1@32, hk2@64, hk3@96; KbufB: hk4@0, hk5@32, hk6@64.
    hk_j = h*k_j directly (sign-positive)."""
    W1 = _np(gen_params[0][0]).astype(np.float64)  # (64, 16)

    def rows_for(coeffs):
        ca = np.zeros((128, 16), dtype=np.float64)
        cb = np.zeros((128, 16), dtype=np.float64)
        ca[0:16, :] = np.eye(16)
        for j, a in enumerate(coeffs):  # j = 0 -> k1
            if j < 3:
                ca[32 * (j + 1):32 * (j + 1) + 16, :] = a * np.eye(16)
            else:
                cb[32 * (j - 3):32 * (j - 3) + 16, :] = a * np.eye(16)
        return ca, cb

    ma_list, mb_list = [], []
    for s in range(6):
        ca, cb = rows_for(A_LOW[s])
        ma_list.append((ca @ W1.T).astype(np.float32))
        mb_list.append((cb @ W1.T).astype(np.float32) if len(A_LOW[s]) > 3 else None)
    cfa, cfb = rows_for(B_W)
    cfa[0:16, :] = np.eye(16)  # y passthrough (already set)
    return ma_list, mb_list, cfa.astype(np.float32), cfb.astype(np.float32)


# 96-row channel-block permutation: row r = 32*c + l (l < 16) <-> orig l*3+c
def _perm96_rows():
    rows = []
    for r in range(96):
        c, l = divmod(r, 32)
        rows.append(l * 3 + c if l < 16 else -1)
    return rows


class _BlobLayout:
    def __init__(self):
        self.cols = 0
        self.slots = {}

    def alloc(self, name, rows, cols):
        self.slots[name] = (rows, self.cols, cols)
        self.cols += cols

    def fill(self, blob, name, arr):
        rows, c0, cols = self.slots[name]
        assert arr.shape == (rows, cols), (name, arr.shape, (rows, cols))
        blob[0:rows, c0:c0 + cols] = arr

    def ap(self, tile, name):
        rows, c0, cols = self.slots[name]
        return tile[0:rows, c0:c0 + cols]


def _make_layout():
    lay = _BlobLayout()
    for s in range(6):
        lay.alloc(f"ma{s}", 128, 64)
    for s in (4, 5):
        lay.alloc(f"mb{s}", 128, 64)
    lay.alloc("cfa", 128, 16)
    lay.alloc("cfb", 128, 16)
    for l in range(4):
        lay.alloc(f"w{l + 2}", 64, 64)
    lay.alloc("w6p", 64, 96)
    lay.alloc("i96", 96, 96)
    lay.alloc("s96", 96, 16)
    lay.alloc("b1", 64, 1)
    for l in range(4):
        lay.alloc(f"b{l + 2}", 64, 1)
    lay.alloc("b6x2", 96, 1)
    lay.alloc("we1", 2, 32)
    for l in range(3):
        lay.alloc(f"we{l + 2}", 32, 32)
    lay.alloc("we5w", 32, 48)   # -> psum rows mu@0-15, pad, logvar@32-47
    for l in range(4):
        lay.alloc(f"be{l + 1}", 32, 1)
    lay.alloc("bmu", 16, 1)
    lay.alloc("blv", 16, 1)
    lay.alloc("blvh", 16, 1)
    lay.alloc("eps0", 16, 64)
    lay.alloc("xs0", 2, 64)
    return lay


def _build_blob(lay, core, enc_params, gen_params, mats, eps0, xs):
    ma_list, mb_list, cfa, cfb = mats
    blob = np.zeros((128, lay.cols), dtype=np.float32)
    for s in range(6):
        lay.fill(blob, f"ma{s}", ma_list[s])
    for s in (4, 5):
        lay.fill(blob, f"mb{s}", mb_list[s])
    lay.fill(blob, "cfa", cfa)
    lay.fill(blob, "cfb", cfb)
    for l in range(4):
        W = _np(gen_params[l + 1][0]).astype(np.float32)
        lay.fill(blob, f"w{l + 2}", W.T.copy())
    W6 = _np(gen_params[5][0]).astype(np.float32)  # (48, 64)
    b6 = _np(gen_params[5][1]).astype(np.float32)
    w6p = np.zeros((64, 96), dtype=np.float32)
    b6x2 = np.zeros((96, 1), dtype=np.float32)
    for r, orig in enumerate(_perm96_rows()):
        if orig >= 0:
            w6p[:, r] = W6[orig]
            b6x2[r, 0] = 2.0 * b6[orig]
    lay.fill(blob, "w6p", w6p)
    lay.fill(blob, "b6x2", b6x2)
    i96 = np.zeros((96, 96), dtype=np.float32)
    for r, orig in enumerate(_perm96_rows()):
        if orig >= 0:
            i96[r, r] = 1.0
    lay.fill(blob, "i96", i96)
    s96 = np.zeros((96, 16), dtype=np.float32)
    for c in range(3):
        for l in range(16):
            s96[32 * c + l, l] = 1.0
    lay.fill(blob, "s96", s96)
    lay.fill(blob, "b1", _np(gen_params[0][1]).astype(np.float32).reshape(64, 1))
    for l in range(4):
        lay.fill(blob, f"b{l + 2}", _np(gen_params[l + 1][1]).astype(np.float32).reshape(64, 1))
    We1 = _np(enc_params[0][0]).astype(np.float32)
    lay.fill(blob, "we1", We1.T.copy())
    for l in range(3):
        lay.fill(blob, f"we{l + 2}", _np(enc_params[l + 1][0]).astype(np.float32).T.copy())
    We5 = _np(enc_params[4][0]).astype(np.float32)  # (32, 32): rows mu 0-15, lv 16-31
    we5w = np.zeros((32, 48), dtype=np.float32)
    we5w[:, 0:16] = We5[0:16].T
    we5w[:, 32:48] = We5[16:32].T
    lay.fill(blob, "we5w", we5w)
    for l in range(4):
        lay.fill(blob, f"be{l + 1}", _np(enc_params[l][1]).astype(np.float32).reshape(32, 1))
    be5 = _np(enc_params[4][1]).astype(np.float32)
    lay.fill(blob, "bmu", be5[0:16].reshape(16, 1))
    lay.fill(blob, "blv", be5[16:32].reshape(16, 1))
    lay.fill(blob, "blvh", (0.5 * be5[16:32]).reshape(16, 1))
    lay.fill(blob, "eps0", eps0[core * BLOC:(core + 1) * BLOC].T.copy())
    lay.fill(blob, "xs0", _np(xs)[core * BLOC:(core + 1) * BLOC, 0, :].astype(np.float32).T.copy())
    return blob


def _build_stream(core, n_steps, eps_all, dc_all, h_arr):
    """(epsT, dcbT): each (n_steps, 96, 384) f32.
    epsT rows 32c+l: STD*eps[l,c]; dcbT rows 32c+l: h*dc[c] (all l).
    Gap rows (l>=16) zero."""
    sl = slice(core * BLOC, (core + 1) * BLOC)
    epsT = np.zeros((n_steps, 96, 384), dtype=np.float32)
    dcbT = np.zeros((n_steps, 96, 384), dtype=np.float32)
    e = eps_all[sl]  # (64, n, 6, 16, 3)
    e = np.transpose(e, (1, 4, 3, 2, 0))  # (n, c, l, e, b)
    for c in range(3):
        epsT[:, 32 * c:32 * c + 16, :] = (np.float32(STD) * e[:, c]).reshape(n_steps, 16, 384)
    d = dc_all[:, :, sl, :]  # (n, 6, 64, 3)
    d = np.transpose(d, (0, 3, 1, 2))  # (n, c, e, b)
    dh = h_arr[:, None, None, None, None].astype(np.float32) * d[:, :, None, :, :]  # (n,c,1,e,b)
    for c in range(3):
        dcbT[:, 32 * c:32 * c + 16, :] = np.broadcast_to(
            dh[:, c], (n_steps, 16, 6, 64)).reshape(n_steps, 16, 384)
    return epsT, dcbT


# --------------------------------------------------------------------------
# Device kernel
# --------------------------------------------------------------------------

def _build_kernel(n_steps, ncols):
    import concourse.bass as bass
    import concourse.bacc as bacc
    import concourse.tile as tile
    from concourse import mybir

    A = mybir.ActivationFunctionType
    ALU = mybir.AluOpType
    f32 = mybir.dt.float32

    nc = bacc.Bacc("TRN2", target_bir_lowering=False, debug=False)

    blob_d = nc.dram_tensor("blob", [128, ncols], f32, kind="ExternalInput")
    eps_d = nc.dram_tensor("epsdata", [n_steps, 96, 384], f32, kind="ExternalInput")
    dcb_d = nc.dram_tensor("dcbdata", [n_steps, 96, 384], f32, kind="ExternalInput")
    zs_d = nc.dram_tensor("zs_out", [T, 16, BLOC], f32, kind="ExternalOutput")
    z0_d = nc.dram_tensor("z0_out", [48, BLOC], f32, kind="ExternalOutput")

    lay = _make_layout()

    with tile.TileContext(nc) as tc:
        with (
            tc.tile_pool(name="consts", bufs=1) as constp,
            tc.tile_pool(name="epsring", bufs=6) as epsp,
            tc.tile_pool(name="dcbring", bufs=6) as dcbp,
            tc.tile_pool(name="acts", bufs=4) as xp,
            tc.tile_pool(name="work", bufs=3) as wp,
            tc.tile_pool(name="zs", bufs=4) as zsp,
            tc.tile_pool(name="ph", bufs=2, space="PSUM") as php,
            tc.tile_pool(name="pe", bufs=2, space="PSUM") as pep,
            tc.tile_pool(name="pmu", bufs=2, space="PSUM") as pmup,
            tc.tile_pool(name="py", bufs=2, space="PSUM") as pyp,
        ):
            blob = constp.tile([128, ncols], f32)
            nc.sync.dma_start(out=blob, in_=blob_d[:, :])
            kbufa = constp.tile([128, 64], f32)
            kbufb = constp.tile([128, 64], f32)
            nc.vector.memset(kbufa, 0.0)
            nc.vector.memset(kbufb, 0.0)

            def act(out, in_, fn, bias=0.0, scale=1.0):
                nc.scalar.activation(out=out, in_=in_, func=fn, bias=bias, scale=scale)

            def bap(name):
                return lay.ap(blob, name)

            # ---------------- encoder ----------------
            pe2 = php.tile([64, 64], f32, tag="ph")
            nc.tensor.matmul(pe2[0:32, :], bap("we1"), bap("xs0"))
            for l in range(4):
                ee = pep.tile([64, 64], f32, tag="ee")
                act(ee[0:32, :], pe2[0:32, :], A.Exp, bias=bap(f"be{l + 1}")[:, 0:1])
                xe = xp.tile([32, 64], f32, tag="xenc")
                act(xe, ee[0:32, :], A.Ln, bias=1.0)
                pe2 = php.tile([64, 64], f32, tag="ph")
                if l < 3:
                    nc.tensor.matmul(pe2[0:32, :], bap(f"we{l + 2}"), xe)
                else:
                    nc.tensor.matmul(pe2[0:48, :], bap("we5w"), xe)
            z0t = wp.tile([48, 64], f32, tag="z0")
            act(z0t[0:16, :], pe2[0:16, :], A.Identity, bias=bap("bmu")[:, 0:1])
            act(z0t[32:48, :], pe2[32:48, :], A.Identity, bias=bap("blv")[:, 0:1])
            stdt = wp.tile([16, 64], f32, tag="stdt")
            act(stdt, pe2[32:48, :], A.Exp, bias=bap("blvh")[:, 0:1], scale=0.5)
            tmpt = wp.tile([16, 64], f32, tag="tmpt")
            nc.vector.tensor_mul(tmpt, bap("eps0"), stdt)
            nc.vector.tensor_add(kbufa[0:16, :], z0t[0:16, :], tmpt)
            nc.sync.dma_start(out=z0_d[:, :], in_=z0t)
            zst0 = zsp.tile([16, 64], f32, tag="zst")
            nc.gpsimd.tensor_copy(out=zst0, in_=kbufa[0:16, :])
            nc.sync.dma_start(out=zs_d[0, :, :], in_=zst0)

            # ---------------- ODE steps ----------------
            for n in range(n_steps):
                epst = epsp.tile([96, 384], f32, tag="eps")
                nc.sync.dma_start(out=epst, in_=eps_d[n, :, :])
                dcbt = dcbp.tile([96, 384], f32, tag="dcb")
                nc.sync.dma_start(out=dcbt, in_=dcb_d[n, :, :])
                for s in range(6):
                    ph = php.tile([64, 64], f32, tag="ph")
                    nc.tensor.matmul(
                        ph, bap(f"ma{s}"), kbufa,
                        start=True, stop=(s < 4),
                    )
                    if s >= 4:
                        nc.tensor.matmul(
                            ph, bap(f"mb{s}"), kbufb, start=False, stop=True
                        )
                    x = None
                    for l in range(5):
                        ee = pep.tile([64, 64], f32, tag="ee")
                        if l == 0:
                            act(ee, ph, A.Exp, bias=bap("b1")[:, 0:1])
                        else:
                            act(ee, ph2, A.Exp, bias=bap(f"b{l + 1}")[:, 0:1])
                        x = xp.tile([64, 64], f32, tag="x")
                        act(x, ee, A.Ln, bias=1.0)
                        if l < 4:
                            ph2 = php.tile([64, 64], f32, tag="ph")
                            nc.tensor.matmul(ph2, bap(f"w{l + 2}"), x)
                    pmu = pmup.tile([96, 64], f32, tag="pmu")
                    nc.tensor.matmul(pmu, bap("w6p"), x, start=True, stop=False)
                    nc.tensor.matmul(
                        pmu, bap("i96"), epst[0:96, s * 64:(s + 1) * 64],
                        start=False, stop=True,
                    )
                    e2z = wp.tile([96, 64], f32, tag="e2z")
                    act(e2z, pmu, A.Exp, bias=bap("b6x2")[:, 0:1], scale=2.0)
                    dd = wp.tile([96, 64], f32, tag="dd")
                    nc.vector.tensor_scalar_add(dd, e2z, 1.0)
                    rr = wp.tile([96, 64], f32, tag="rr")
                    nc.vector.reciprocal_approx_fast(out=rr, in_=dd)
                    # prod = (1 - 2r) * (h*dc), fused in one custom-DVE op
                    prod = wp.tile([96, 64], f32, tag="prod")
                    pacc = wp.tile([96, 1], f32, tag="pacc")
                    nc.vector.affine_mul_reduce(
                        out=prod, accum_out=pacc, in0=rr,
                        in1=dcbt[0:96, s * 64:(s + 1) * 64],
                        scale=-2.0, bias=1.0,
                    )
                    # h*k_{s+1} = sum of the three 32-aligned channel blocks,
                    # via a block-sum matmul, landing in its Kbuf slot
                    pk = pyp.tile([16, 64], f32, tag="py")
                    nc.tensor.matmul(pk, bap("s96"), prod)
                    if s < 3:
                        dst = kbufa[32 * (s + 1):32 * (s + 1) + 16, :]
                    else:
                        dst = kbufb[32 * (s - 3):32 * (s - 3) + 16, :]
                    nc.vector.tensor_copy(out=dst, in_=pk)
                py = pyp.tile([16, 64], f32, tag="py")
                nc.tensor.matmul(py, bap("cfa"), kbufa, start=True, stop=False)
                nc.tensor.matmul(py, bap("cfb"), kbufb, start=False, stop=True)
                nc.vector.tensor_copy(out=kbufa[0:16, :], in_=py)
                zst = zsp.tile([16, 64], f32, tag="zst")
                nc.gpsimd.tensor_copy(out=zst, in_=kbufa[0:16, :])
                nc.sync.dma_start(out=zs_d[n + 1, :, :], in_=zst)

    nc.compile()
    return nc


_KERNEL_CACHE = {}


def _get_kernel(n_steps, ncols):
    key = (n_steps, ncols)
    if key not in _KERNEL_CACHE:
        _KERNEL_CACHE[key] = _build_kernel(n_steps, ncols)
    return _KERNEL_CACHE[key]


def kernel(xs, key, enc_params, gen_params, dec_params, _n_steps=None, _collect=None):
    """Full-input entry point. Returns (x_recons, zs, (z0_mu, z0_logvar))."""
    from concourse.bass_utils import run_bass_kernel_spmd

    xs = _np(xs)
    key_np = _np(key)
    n_steps = NSTEPS_FULL if _n_steps is None else _n_steps

    eps0, eps_all, dc_all, h_arr = _host_precompute(xs, key_np, n_steps)
    mats = _stage_matrices(gen_params)
    lay = _make_layout()

    in_maps = []
    for core in range(NCORES):
        blob = _build_blob(lay, core, enc_params, gen_params, mats, eps0, xs)
        epsT, dcbT = _build_stream(core, n_steps, eps_all, dc_all, h_arr)
        in_maps.append({"blob": blob, "epsdata": epsT, "dcbdata": dcbT})

    nc = _get_kernel(n_steps, lay.cols)
    import time as _time
    _t0 = _time.time()
    res = run_bass_kernel_spmd(
        nc, in_maps, core_ids=list(range(NCORES)),
        trace=bool(int(os.environ.get("NODE_TRACE", "0"))),
    )
    _exec_wall = _time.time() - _t0
    if _collect is not None:
        _collect["results"] = res
        _collect["exec_wall_s"] = _exec_wall

    zs = np.zeros((B, T, L), dtype=np.float32)
    z0_mu = np.zeros((B, L), dtype=np.float32)
    z0_logvar = np.zeros((B, L), dtype=np.float32)
    for core in range(NCORES):
        r = res.results[core]
        sl = slice(core * BLOC, (core + 1) * BLOC)
        zs[sl] = np.transpose(r["zs_out"], (2, 0, 1))  # (T,16,64)->(64,T,16)
        z0_mu[sl] = r["z0_out"][0:16].T
        z0_logvar[sl] = r["z0_out"][32:48].T

    Wd = _np(dec_params[0]).astype(np.float32)
    bd = _np(dec_params[1]).astype(np.float32)
    x_recons = zs @ Wd.T + bd
    return x_recons, zs, (z0_mu, z0_logvar)


# revision 10
# speedup vs baseline: 2.8716x; 2.8716x over previous
"""Trainium2 Bass kernel for nn_NeuralODE_7121055776945.

Latent neural ODE: encoder MLP -> variational z0 -> 499-step fixed-step
Tsit5 solve; each of the 6 stages evaluates a 6-layer MLP (16->64x5->48),
perturbs with per-sample threefry noise, applies tanh and contracts with a
cubic-Hermite control derivative.

Strategy:
  - Pure data parallel: batch 512 -> 64 samples per core on 8 cores.
  - Feature-major on-chip layout: activations are (features, batch) tiles.
  - The RK stage combination y_s = y + h*sum_j a_sj*k_j is fused into the
    first MLP matmul: lhsT = Cs @ W1.T precomputed on host, contracting two
    128-partition "Kbuf" tiles holding y and the h*k_j stage results at
    32-aligned partition slots (compute-engine APs must start at partition
    0/32/64/96 on TRN2).
  - softplus(x) = ln(1 + e^x) via two ScalarE ops (exp, then ln with bias=1);
    both live in the natural_log_exp_and_others activation table set so the
    act table loads exactly once (this build has no native Softplus table).
  - tanh(z) = 1 - 2/(1+e^{2z}): ScalarE exp(scale=2, bias=2*b6) reading the
    matmul PSUM, VectorE +1, custom-DVE fast reciprocal (~51 ULP), then one
    fused tensor_scalar (r*-2+1).  The per-eval channel blocks live at rows
    {0,32,64} of 96-row tiles so the partition-block reduction uses legal
    32-aligned slices.  h is folded into the streamed dcontrol data
    (dcb = h*dc), so per-step h values are exact.
  - Per-sample threefry RNG (eps0 and all 2994 stage-eval eps) and the
    cubic-Hermite dcontrol values are bit-exactly precomputed on host with
    jax on CPU, replicating the reference's exact vmap/scan structure (jax
    RNG values depend on the batching structure!).

All matmuls are fp32 (bf16/tf32 fail the accuracy budget: this ODE amplifies
activation-level noise ~80x over the 499 steps).
"""

import os
import numpy as np

B, T, D, L, H = 512, 500, 2, 16, 32
STD = 1e-3
NCORES = 8
BLOC = B // NCORES  # 64
NSTEPS_FULL = T - 1  # 499

C2, C3, C4, C5 = 0.161, 0.327, 0.9, 0.9800255409045097
A_LOW = [
    [],
    [0.161],
    [-0.008480655492356989, 0.335480655492357],
    [2.8971530571054935, -6.359448489975075, 4.3622954328695815],
    [5.325864828439257, -11.748883564062828, 7.4955393428898365, -0.09249506636175525],
    [5.86145544294642, -12.92096931784711, 8.159367898576159, -0.071584973281401, -0.028269050394068383],
]
B_W = [0.09646076681806523, 0.01, 0.4798896504144996, 1.379008574103742, -3.290069515436081, 2.324710524099774]

# Kbuf slot layout: KbufA rows {0: y, 32: hk1, 64: hk2, 96: hk3},
# KbufB rows {0: hk4, 32: hk5, 64: hk6}.  All other rows zero.
SLOT_A = {0: 0, 1: 32, 2: 64, 3: 96}   # y, hk1..hk3
SLOT_B = {4: 0, 5: 32, 6: 64}          # hk4..hk6 (key = j index 1-based - 3?)


def _np(x):
    return np.asarray(x)


# --------------------------------------------------------------------------
# Host-side exact precompute (jax on CPU)
# --------------------------------------------------------------------------

def _host_precompute(xs, key, n_steps):
    """Returns (eps0 (B,L) f32, eps_all (B, n_steps, 6, L, D+1) f32,
    dc_all (n_steps, 6, B, D+1) f32, h_arr (n_steps,) f32)."""
    import jax
    import jax.numpy as jnp

    cpu = jax.devices("cpu")[0]
    with jax.default_device(cpu):
        xs_j = jnp.asarray(xs, dtype=jnp.float32)
        key_j = jnp.asarray(key)
        t_eval = jnp.linspace(0.0, 1.0, T, dtype=jnp.float32)
        dt = t_eval[1:] - t_eval[:-1]

        keys = jax.random.split(key_j, B)

        @jax.jit
        def eps0_fn(keys):
            return jax.vmap(lambda k: jax.random.normal(k, (L,)))(keys)

        eps0 = np.asarray(eps0_fn(keys), dtype=np.float32)

        t0 = t_eval[:n_steps]
        h = dt[:n_steps]

        # eps must replicate the reference's exact computation structure:
        # jax.random values differ between batched (vmap-over-keys) and
        # unbatched calls, and also depend on the vmap nesting. The reference
        # calls vmap(lambda k: normal(fold_in(k, ti), (L, D+1)))(keys) once
        # per stage inside a scan; mirror exactly that.
        @jax.jit
        def eps_scan(keys, t0s, hs):
            def body(carry, th):
                tt0, hh = th
                outs = []
                for tt in (tt0, tt0 + C2 * hh, tt0 + C3 * hh, tt0 + C4 * hh,
                           tt0 + C5 * hh, tt0 + hh):
                    ti = (tt * 1e9).astype(jnp.int32)
                    outs.append(jax.vmap(
                        lambda k: jax.random.normal(
                            jax.random.fold_in(k, ti), (L, D + 1))
                    )(keys))
                return carry, jnp.stack(outs)

            _, out = jax.lax.scan(body, 0, (t0s, hs))
            return out  # (n_steps, 6, B, L, D+1)

        eps_all = np.asarray(eps_scan(keys, t0, h), dtype=np.float32)
        eps_all = np.ascontiguousarray(np.transpose(eps_all, (2, 0, 1, 3, 4)))
        # (B, n_steps, 6, L, D+1)

        tmat = jnp.stack(
            [t0, t0 + C2 * h, t0 + C3 * h, t0 + C4 * h, t0 + C5 * h, t0 + h], axis=1
        )  # (n_steps, 6) f32

        xs_aug = jnp.concatenate(
            [jnp.broadcast_to(t_eval[None, :, None], (B, T, 1)), xs_j], axis=-1
        )
        slopes = (xs_aug[:, 1:] - xs_aug[:, :-1]) / dt[None, :, None]
        ds = jnp.concatenate([slopes[:, :1], slopes], axis=1)

        @jax.jit
        def dc_fn(ts):
            def dcontrol(t):
                i = jnp.clip(jnp.searchsorted(t_eval, t, side="right") - 1, 0, T - 2)
                hh = t_eval[i + 1] - t_eval[i]
                s = (t - t_eval[i]) / hh
                y_l = jnp.take(xs_aug, i, axis=1)
                y_r = jnp.take(xs_aug, i + 1, axis=1)
                d_l = jnp.take(ds, i, axis=1)
                d_r = jnp.take(ds, i + 1, axis=1)
                return (
                    ((6 * s * s - 6 * s) * (y_l - y_r)) / hh
                    + (3 * s * s - 4 * s + 1) * d_l
                    + (3 * s * s - 2 * s) * d_r
                )
            return jax.vmap(dcontrol)(ts)

        dc_all = np.asarray(dc_fn(tmat.reshape(-1)), dtype=np.float32)
        dc_all = dc_all.reshape(n_steps, 6, B, D + 1)
        h_arr = np.asarray(h, dtype=np.float32)

    return eps0, eps_all, dc_all, h_arr


def _stage_matrices(gen_params):
    """Fused first-layer weights.  Returns (ma_list[6] (128,64), mb_list[6]
    (128,64) or None, cfa (128,16), cfb (128,16)), fp32.
    KbufA rows: y@0, hk1@32, hk2@64, hk3@96; KbufB: hk4@0, hk5@32, hk6@64.
    hk_j = h*k_j directly (sign-positive)."""
    W1 = _np(gen_params[0][0]).astype(np.float64)  # (64, 16)

    def rows_for(coeffs):
        ca = np.zeros((128, 16), dtype=np.float64)
        cb = np.zeros((128, 16), dtype=np.float64)
        ca[0:16, :] = np.eye(16)
        for j, a in enumerate(coeffs):  # j = 0 -> k1
            if j < 3:
                ca[32 * (j + 1):32 * (j + 1) + 16, :] = a * np.eye(16)
            else:
                cb[32 * (j - 3):32 * (j - 3) + 16, :] = a * np.eye(16)
        return ca, cb

    ma_list, mb_list = [], []
    for s in range(6):
        ca, cb = rows_for(A_LOW[s])
        ma_list.append((ca @ W1.T).astype(np.float32))
        mb_list.append((cb @ W1.T).astype(np.float32) if len(A_LOW[s]) > 3 else None)
    cfa, cfb = rows_for(B_W)
    cfa[0:16, :] = np.eye(16)  # y passthrough (already set)
    return ma_list, mb_list, cfa.astype(np.float32), cfb.astype(np.float32)


# 96-row channel-block permutation: row r = 32*c + l (l < 16) <-> orig l*3+c
def _perm96_rows():
    rows = []
    for r in range(96):
        c, l = divmod(r, 32)
        rows.append(l * 3 + c if l < 16 else -1)
    return rows


class _BlobLayout:
    def __init__(self):
        self.cols = 0
        self.slots = {}

    def alloc(self, name, rows, cols):
        self.slots[name] = (rows, self.cols, cols)
        self.cols += cols

    def fill(self, blob, name, arr):
        rows, c0, cols = self.slots[name]
        assert arr.shape == (rows, cols), (name, arr.shape, (rows, cols))
        blob[0:rows, c0:c0 + cols] = arr

    def ap(self, tile, name):
        rows, c0, cols = self.slots[name]
        return tile[0:rows, c0:c0 + cols]


def _make_layout():
    lay = _BlobLayout()
    for s in range(6):
        lay.alloc(f"ma{s}", 128, 64)
    for s in (4, 5):
        lay.alloc(f"mb{s}", 128, 64)
    lay.alloc("cfa", 128, 16)
    lay.alloc("cfb", 128, 16)
    for l in range(4):
        lay.alloc(f"w{l + 2}", 64, 64)
    lay.alloc("w6p", 64, 96)
    lay.alloc("i96", 96, 96)
    lay.alloc("s96", 96, 16)
    lay.alloc("b1", 64, 1)
    for l in range(4):
        lay.alloc(f"b{l + 2}", 64, 1)
    lay.alloc("b6x2", 96, 1)
    lay.alloc("we1", 2, 32)
    for l in range(3):
        lay.alloc(f"we{l + 2}", 32, 32)
    lay.alloc("we5w", 32, 48)   # -> psum rows mu@0-15, pad, logvar@32-47
    for l in range(4):
        lay.alloc(f"be{l + 1}", 32, 1)
    lay.alloc("bmu", 16, 1)
    lay.alloc("blv", 16, 1)
    lay.alloc("blvh", 16, 1)
    lay.alloc("eps0", 16, 64)
    lay.alloc("xs0", 2, 64)
    return lay


def _build_blob(lay, core, enc_params, gen_params, mats, eps0, xs):
    ma_list, mb_list, cfa, cfb = mats
    blob = np.zeros((128, lay.cols), dtype=np.float32)
    for s in range(6):
        lay.fill(blob, f"ma{s}", ma_list[s])
    for s in (4, 5):
        lay.fill(blob, f"mb{s}", mb_list[s])
    lay.fill(blob, "cfa", cfa)
    lay.fill(blob, "cfb", cfb)
    for l in range(4):
        W = _np(gen_params[l + 1][0]).astype(np.float32)
        lay.fill(blob, f"w{l + 2}", W.T.copy())
    W6 = _np(gen_params[5][0]).astype(np.float32)  # (48, 64)
    b6 = _np(gen_params[5][1]).astype(np.float32)
    w6p = np.zeros((64, 96), dtype=np.float32)
    b6x2 = np.zeros((96, 1), dtype=np.float32)
    for r, orig in enumerate(_perm96_rows()):
        if orig >= 0:
            w6p[:, r] = W6[orig]
            b6x2[r, 0] = 2.0 * b6[orig]
    lay.fill(blob, "w6p", w6p)
    lay.fill(blob, "b6x2", b6x2)
    i96 = np.zeros((96, 96), dtype=np.float32)
    for r, orig in enumerate(_perm96_rows()):
        if orig >= 0:
            i96[r, r] = 1.0
    lay.fill(blob, "i96", i96)
    s96 = np.zeros((96, 16), dtype=np.float32)
    for c in range(3):
        for l in range(16):
            s96[32 * c + l, l] = 1.0
    lay.fill(blob, "s96", s96)
    lay.fill(blob, "b1", _np(gen_params[0][1]).astype(np.float32).reshape(64, 1))
    for l in range(4):
        lay.fill(blob, f"b{l + 2}", _np(gen_params[l + 1][1]).astype(np.float32).reshape(64, 1))
    We1 = _np(enc_params[0][0]).astype(np.float32)
    lay.fill(blob, "we1", We1.T.copy())
    for l in range(3):
        lay.fill(blob, f"we{l + 2}", _np(enc_params[l + 1][0]).astype(np.float32).T.copy())
    We5 = _np(enc_params[4][0]).astype(np.float32)  # (32, 32): rows mu 0-15, lv 16-31
    we5w = np.zeros((32, 48), dtype=np.float32)
    we5w[:, 0:16] = We5[0:16].T
    we5w[:, 32:48] = We5[16:32].T
    lay.fill(blob, "we5w", we5w)
    for l in range(4):
        lay.fill(blob, f"be{l + 1}", _np(enc_params[l][1]).astype(np.float32).reshape(32, 1))
    be5 = _np(enc_params[4][1]).astype(np.float32)
    lay.fill(blob, "bmu", be5[0:16].reshape(16, 1))
    lay.fill(blob, "blv", be5[16:32].reshape(16, 1))
    lay.fill(blob, "blvh", (0.5 * be5[16:32]).reshape(16, 1))
    lay.fill(blob, "eps0", eps0[core * BLOC:(core + 1) * BLOC].T.copy())
    lay.fill(blob, "xs0", _np(xs)[core * BLOC:(core + 1) * BLOC, 0, :].astype(np.float32).T.copy())
    return blob


def _build_stream(core, n_steps, eps_all, dc_all, h_arr):
    """(epsT, dcbT): each (n_steps, 96, 384) f32.
    epsT rows 32c+l: STD*eps[l,c]; dcbT rows 32c+l: h*dc[c] (all l).
    Gap rows (l>=16) zero."""
    sl = slice(core * BLOC, (core + 1) * BLOC)
    epsT = np.zeros((n_steps, 96, 384), dtype=np.float32)
    dcbT = np.zeros((n_steps, 96, 384), dtype=np.float32)
    e = eps_all[sl]  # (64, n, 6, 16, 3)
    e = np.transpose(e, (1, 4, 3, 2, 0))  # (n, c, l, e, b)
    for c in range(3):
        epsT[:, 32 * c:32 * c + 16, :] = (np.float32(STD) * e[:, c]).reshape(n_steps, 16, 384)
    d = dc_all[:, :, sl, :]  # (n, 6, 64, 3)
    d = np.transpose(d, (0, 3, 1, 2))  # (n, c, e, b)
    dh = h_arr[:, None, None, None, None].astype(np.float32) * d[:, :, None, :, :]  # (n,c,1,e,b)
    for c in range(3):
        dcbT[:, 32 * c:32 * c + 16, :] = np.broadcast_to(
            dh[:, c], (n_steps, 16, 6, 64)).reshape(n_steps, 16, 384)
    return epsT, dcbT


# --------------------------------------------------------------------------
# Device kernel
# --------------------------------------------------------------------------

def _build_kernel(n_steps, ncols):
    import concourse.bass as bass
    import concourse.bacc as bacc
    import concourse.tile as tile
    from concourse import mybir

    A = mybir.ActivationFunctionType
    ALU = mybir.AluOpType
    f32 = mybir.dt.float32

    # Force all ScalarE activations (Exp/Ln/Identity/Copy) onto the single
    # natural_log_exp_and_others table set: the act-table placement pass
    # otherwise assigns exp and ln to different sets and inserts a ~2.7us
    # table reload around nearly every activation (~60 reloads per ODE step).
    # Mutating the cached dict in place keeps set indices aligned with
    # act_info.json, which walrus requires.
    import concourse.hw_specs as hw_specs
    _tabs = hw_specs.get_activation_tables("gen3")
    _target = "natural_log_exp_and_others"
    if _target in _tabs:
        _strip = {A.Exp, A.Ln, A.Identity, A.Copy}
        for _name, _funcs in _tabs.items():
            if _name != _target:
                _funcs -= _strip

    nc = bacc.Bacc("TRN2", target_bir_lowering=False, debug=False)

    blob_d = nc.dram_tensor("blob", [128, ncols], f32, kind="ExternalInput")
    eps_d = nc.dram_tensor("epsdata", [n_steps, 96, 384], f32, kind="ExternalInput")
    dcb_d = nc.dram_tensor("dcbdata", [n_steps, 96, 384], f32, kind="ExternalInput")
    zs_d = nc.dram_tensor("zs_out", [T, 16, BLOC], f32, kind="ExternalOutput")
    z0_d = nc.dram_tensor("z0_out", [48, BLOC], f32, kind="ExternalOutput")

    lay = _make_layout()

    with tile.TileContext(nc) as tc:
        with (
            tc.tile_pool(name="consts", bufs=1) as constp,
            tc.tile_pool(name="epsring", bufs=6) as epsp,
            tc.tile_pool(name="dcbring", bufs=6) as dcbp,
            tc.tile_pool(name="acts", bufs=4) as xp,
            tc.tile_pool(name="work", bufs=3) as wp,
            tc.tile_pool(name="zs", bufs=4) as zsp,
            tc.tile_pool(name="ph", bufs=2, space="PSUM") as php,
            tc.tile_pool(name="pe", bufs=2, space="PSUM") as pep,
            tc.tile_pool(name="pmu", bufs=2, space="PSUM") as pmup,
            tc.tile_pool(name="py", bufs=2, space="PSUM") as pyp,
        ):
            blob = constp.tile([128, ncols], f32)
            nc.sync.dma_start(out=blob, in_=blob_d[:, :])
            kbufa = constp.tile([128, 64], f32)
            kbufb = constp.tile([128, 64], f32)
            nc.vector.memset(kbufa, 0.0)
            nc.vector.memset(kbufb, 0.0)

            def act(out, in_, fn, bias=0.0, scale=1.0):
                nc.scalar.activation(out=out, in_=in_, func=fn, bias=bias, scale=scale)

            def bap(name):
                return lay.ap(blob, name)

            # ---------------- encoder ----------------
            pe2 = php.tile([64, 64], f32, tag="ph")
            nc.tensor.matmul(pe2[0:32, :], bap("we1"), bap("xs0"))
            for l in range(4):
                ee = pep.tile([64, 64], f32, tag="ee")
                act(ee[0:32, :], pe2[0:32, :], A.Exp, bias=bap(f"be{l + 1}")[:, 0:1])
                xe = xp.tile([32, 64], f32, tag="xenc")
                act(xe, ee[0:32, :], A.Ln, bias=1.0)
                pe2 = php.tile([64, 64], f32, tag="ph")
                if l < 3:
                    nc.tensor.matmul(pe2[0:32, :], bap(f"we{l + 2}"), xe)
                else:
                    nc.tensor.matmul(pe2[0:48, :], bap("we5w"), xe)
            z0t = wp.tile([48, 64], f32, tag="z0")
            act(z0t[0:16, :], pe2[0:16, :], A.Identity, bias=bap("bmu")[:, 0:1])
            act(z0t[32:48, :], pe2[32:48, :], A.Identity, bias=bap("blv")[:, 0:1])
            stdt = wp.tile([16, 64], f32, tag="stdt")
            act(stdt, pe2[32:48, :], A.Exp, bias=bap("blvh")[:, 0:1], scale=0.5)
            tmpt = wp.tile([16, 64], f32, tag="tmpt")
            nc.vector.tensor_mul(tmpt, bap("eps0"), stdt)
            nc.vector.tensor_add(kbufa[0:16, :], z0t[0:16, :], tmpt)
            nc.sync.dma_start(out=z0_d[:, :], in_=z0t)
            zst0 = zsp.tile([16, 64], f32, tag="zst")
            nc.gpsimd.tensor_copy(out=zst0, in_=kbufa[0:16, :])
            nc.sync.dma_start(out=zs_d[0, :, :], in_=zst0)

            # ---------------- ODE steps ----------------
            for n in range(n_steps):
                epst = epsp.tile([96, 384], f32, tag="eps")
                nc.sync.dma_start(out=epst, in_=eps_d[n, :, :])
                dcbt = dcbp.tile([96, 384], f32, tag="dcb")
                nc.sync.dma_start(out=dcbt, in_=dcb_d[n, :, :])
                for s in range(6):
                    ph = php.tile([64, 64], f32, tag="ph")
                    nc.tensor.matmul(
                        ph, bap(f"ma{s}"), kbufa,
                        start=True, stop=(s < 4),
                    )
                    if s >= 4:
                        nc.tensor.matmul(
                            ph, bap(f"mb{s}"), kbufb, start=False, stop=True
                        )
                    x = None
                    for l in range(5):
                        ee = pep.tile([64, 64], f32, tag="ee")
                        if l == 0:
                            act(ee, ph, A.Exp, bias=bap("b1")[:, 0:1])
                        else:
                            act(ee, ph2, A.Exp, bias=bap(f"b{l + 1}")[:, 0:1])
                        x = xp.tile([64, 64], f32, tag="x")
                        act(x, ee, A.Ln, bias=1.0)
                        if l < 4:
                            ph2 = php.tile([64, 64], f32, tag="ph")
                            nc.tensor.matmul(ph2, bap(f"w{l + 2}"), x)
                    pmu = pmup.tile([96, 64], f32, tag="pmu")
                    nc.tensor.matmul(pmu, bap("w6p"), x, start=True, stop=False)
                    nc.tensor.matmul(
                        pmu, bap("i96"), epst[0:96, s * 64:(s + 1) * 64],
                        start=False, stop=True,
                    )
                    e2z = wp.tile([96, 64], f32, tag="e2z")
                    act(e2z, pmu, A.Exp, bias=bap("b6x2")[:, 0:1], scale=2.0)
                    dd = wp.tile([96, 64], f32, tag="dd")
                    nc.vector.tensor_scalar_add(dd, e2z, 1.0)
                    rr = wp.tile([96, 64], f32, tag="rr")
                    nc.vector.reciprocal_approx_fast(out=rr, in_=dd)
                    # prod = (1 - 2r) * (h*dc), fused in one custom-DVE op
                    prod = wp.tile([96, 64], f32, tag="prod")
                    pacc = wp.tile([96, 1], f32, tag="pacc")
                    nc.vector.affine_mul_reduce(
                        out=prod, accum_out=pacc, in0=rr,
                        in1=dcbt[0:96, s * 64:(s + 1) * 64],
                        scale=-2.0, bias=1.0,
                    )
                    # h*k_{s+1} = sum of the three 32-aligned channel blocks,
                    # via a block-sum matmul, landing in its Kbuf slot
                    pk = pyp.tile([16, 64], f32, tag="py")
                    nc.tensor.matmul(pk, bap("s96"), prod)
                    if s < 3:
                        dst = kbufa[32 * (s + 1):32 * (s + 1) + 16, :]
                    else:
                        dst = kbufb[32 * (s - 3):32 * (s - 3) + 16, :]
                    nc.vector.tensor_copy(out=dst, in_=pk)
                py = pyp.tile([16, 64], f32, tag="py")
                nc.tensor.matmul(py, bap("cfa"), kbufa, start=True, stop=False)
                nc.tensor.matmul(py, bap("cfb"), kbufb, start=False, stop=True)
                nc.vector.tensor_copy(out=kbufa[0:16, :], in_=py)
                zst = zsp.tile([16, 64], f32, tag="zst")
                nc.gpsimd.tensor_copy(out=zst, in_=kbufa[0:16, :])
                nc.sync.dma_start(out=zs_d[n + 1, :, :], in_=zst)

    nc.compile()
    return nc


_KERNEL_CACHE = {}


def _get_kernel(n_steps, ncols):
    key = (n_steps, ncols)
    if key not in _KERNEL_CACHE:
        _KERNEL_CACHE[key] = _build_kernel(n_steps, ncols)
    return _KERNEL_CACHE[key]


def kernel(xs, key, enc_params, gen_params, dec_params, _n_steps=None, _collect=None):
    """Full-input entry point. Returns (x_recons, zs, (z0_mu, z0_logvar))."""
    from concourse.bass_utils import run_bass_kernel_spmd

    xs = _np(xs)
    key_np = _np(key)
    n_steps = NSTEPS_FULL if _n_steps is None else _n_steps

    eps0, eps_all, dc_all, h_arr = _host_precompute(xs, key_np, n_steps)
    mats = _stage_matrices(gen_params)
    lay = _make_layout()

    in_maps = []
    for core in range(NCORES):
        blob = _build_blob(lay, core, enc_params, gen_params, mats, eps0, xs)
        epsT, dcbT = _build_stream(core, n_steps, eps_all, dc_all, h_arr)
        in_maps.append({"blob": blob, "epsdata": epsT, "dcbdata": dcbT})

    nc = _get_kernel(n_steps, lay.cols)
    import time as _time
    _t0 = _time.time()
    res = run_bass_kernel_spmd(
        nc, in_maps, core_ids=list(range(NCORES)),
        trace=bool(int(os.environ.get("NODE_TRACE", "0"))),
    )
    _exec_wall = _time.time() - _t0
    if _collect is not None:
        _collect["results"] = res
        _collect["exec_wall_s"] = _exec_wall

    zs = np.zeros((B, T, L), dtype=np.float32)
    z0_mu = np.zeros((B, L), dtype=np.float32)
    z0_logvar = np.zeros((B, L), dtype=np.float32)
    for core in range(NCORES):
        r = res.results[core]
        sl = slice(core * BLOC, (core + 1) * BLOC)
        zs[sl] = np.transpose(r["zs_out"], (2, 0, 1))  # (T,16,64)->(64,T,16)
        z0_mu[sl] = r["z0_out"][0:16].T
        z0_logvar[sl] = r["z0_out"][32:48].T

    Wd = _np(dec_params[0]).astype(np.float32)
    bd = _np(dec_params[1]).astype(np.float32)
    x_recons = zs @ Wd.T + bd
    return x_recons, zs, (z0_mu, z0_logvar)
